# revision 10
# baseline (speedup 1.0000x reference)
"""Causal self-attention on 8 Trainium2 NeuronCores (Bass/Tile).

Problem: x[4, 2048, 1024], w_in[3072, 1024], w_out[1024, 1024], 16 heads.
    qkv = x @ w_in.T ; per-(b,h) causal softmax attention ; out = y @ w_out.T

Sharding (SPMD — one program, per-core input data):
    core c  ->  batch b = c // 2, head-group g = c % 2 (heads 8g .. 8g+7).
    Each core projects q/k/v for its 8 heads of its batch and runs causal
    attention for them.  The pair (2b, 2b+1) AllGathers the two head-group
    halves of yT, then each core computes the output projection for half of
    the output features (core even: e_out 0..511, odd: 512..1023) over all
    2048 tokens of its batch.  The host concatenates.

Everything on-chip is kept feature-major ("T" = contraction dim on SBUF
partitions) so no on-device transposes are needed:
    xT [D, S] (host-transposed), qT/kT per head-pair [128, S],
    scoresT [k, q], yT [e, t], outT [e_out, t] (host-transposed back).
Softmax denominators come from a ones-column appended to V (the AV matmul
has M = 65: 64 y-dims + the running sum); normalization is a fast DVE
reciprocal on the sum row + a K=1 matmul broadcast + one multiply.
Matmuls run as float32r (full PE rate at N >= 256); storage is fp32.
"""

import sys

for _p in ("/opt/trn_rl_repo",):
    if _p not in sys.path:
        sys.path.insert(0, _p)

import numpy as np

B, S, D = 4, 2048, 1024
H, HD = 16, 64
N_CORES = 8
HPC = 8           # heads per core
NPAIRS = HPC // 2  # head pairs per core
QC = S // 512     # q-chunks per head
TT = S // 128     # token tiles
DT = D // 128     # feature (d) tiles
EHALF = D // 2    # output features per core

_PROG = None      # cached compiled program


def _build_program():
    import concourse.bass as bass
    from concourse import bacc
    import concourse.tile as tile
    import concourse.mybir as mybir

    f32 = mybir.dt.float32
    f32r = mybir.dt.float32r
    AF = mybir.ActivationFunctionType
    OP = mybir.AluOpType

    nc = bacc.Bacc("TRN2", target_bir_lowering=False, debug=False,
                   num_devices=N_CORES)

    xT = nc.dram_tensor("xT", [D, S], f32r, kind="ExternalInput").ap()
    wqkT = nc.dram_tensor("wqkT", [D, 2 * HPC * HD], f32r,
                          kind="ExternalInput").ap()
    wvT = nc.dram_tensor("wvT", [D, HPC * HD], f32r, kind="ExternalInput").ap()
    woT = nc.dram_tensor("woT", [D, EHALF], f32r, kind="ExternalInput").ap()
    tri = nc.dram_tensor("tri", [128, 128], f32, kind="ExternalInput").ap()
    outT = nc.dram_tensor("outT", [EHALF, S], f32, kind="ExternalOutput").ap()

    y_loc = nc.dram_tensor("y_loc", [HPC * HD, S], f32r)
    y_gat = nc.dram_tensor("y_gat", [2, HPC * HD, S], f32r)

    from contextlib import ExitStack

    with tile.TileContext(nc) as tc:
        def mm(out, lhsT, rhs, start, stop):
            nc.tensor.matmul(out, lhsT, rhs, start=start, stop=stop)

        with ExitStack() as perm:
            const_pool = perm.enter_context(tc.tile_pool(name="const", bufs=1))
            v_pool = perm.enter_context(tc.tile_pool(name="vsb", bufs=TT))
            mm_ps = perm.enter_context(
                tc.tile_pool(name="mmps", bufs=2, space="PSUM"))

            tri_sb = const_pool.tile([128, 128], f32, tag="tri")
            nc.sync.dma_start(tri_sb[:], tri[:])
            ones_sb = const_pool.tile([128, 64], f32, tag="ones")
            nc.gpsimd.memset(ones_sb[:], 1.0)

            # v_sb[t]: [128, 8*65] — per head 64 v-columns + a ones column
            v_sb = [v_pool.tile([128, HPC * (HD + 1)], f32r, tag="v", name=f"v{t}")
                    for t in range(TT)]

            with ExitStack() as att_scope:
                qk_pool = att_scope.enter_context(
                    tc.tile_pool(name="qksb", bufs=2 * NPAIRS))
                q_sb = [qk_pool.tile([128, S], f32r, tag="qk", name=f"q{i}")
                        for i in range(NPAIRS)]
                k_sb = [qk_pool.tile([128, S], f32r, tag="qk", name=f"k{i}")
                        for i in range(NPAIRS)]

                # ---------------- phase 1: projections ----------------
                with ExitStack() as proj:
                    xt_pool = proj.enter_context(
                        tc.tile_pool(name="xtsb", bufs=DT))
                    xt_sb = [xt_pool.tile([128, S], f32r, tag="xt", name=f"xt{d}")
                             for d in range(DT)]
                    for d in range(DT):
                        nc.sync.dma_start(xt_sb[d][:],
                                          xT[d * 128:(d + 1) * 128, :])

                    # v[t, e] accumulated over d  (natural layout: k on rows)
                    with tc.tile_pool(name="wvsb", bufs=DT) as wv_pool:
                        wv_sb = [wv_pool.tile([128, HPC * HD], f32r, tag="wv", name=f"wv{d}")
                                 for d in range(DT)]
                        for d in range(DT):
                            nc.sync.dma_start(wv_sb[d][:],
                                              wvT[d * 128:(d + 1) * 128, :])
                        for t in range(TT):
                            ps = mm_ps.tile([128, 512], f32, tag="mm")
                            for d in range(DT):
                                mm(ps[:],
                                   xt_sb[d][:, t * 128:(t + 1) * 128],
                                   wv_sb[d][:],
                                   start=(d == 0), stop=(d == DT - 1))
                            dst = v_sb[t][:].rearrange(
                                "p (h e) -> p h e", h=HPC)[:, :, 0:HD]
                            src = ps[:].rearrange("p (h e) -> p h e", h=HPC)
                            nc.vector.tensor_copy(dst, src)
                            nc.vector.tensor_copy(
                                v_sb[t][:].rearrange(
                                    "p (h e) -> p h e", h=HPC)[:, :, HD:HD + 1],
                                ones_sb[:, 0:HPC].unsqueeze(-1))

                    # qT/kT per pair: [e, t] with e on partitions
                    with tc.tile_pool(name="wqksb", bufs=DT) as wqk_pool:
                        for i in range(NPAIRS):
                            wqk_sb = [wqk_pool.tile([128, 256], f32r, tag="wqk", name=f"wqk{i}_{d}")
                                      for d in range(DT)]
                            for d in range(DT):
                                nc.sync.dma_start(
                                    wqk_sb[d][:, 0:128],
                                    wqkT[d * 128:(d + 1) * 128,
                                         i * 128:(i + 1) * 128])
                                nc.sync.dma_start(
                                    wqk_sb[d][:, 128:256],
                                    wqkT[d * 128:(d + 1) * 128,
                                         (NPAIRS + i) * 128:
                                         (NPAIRS + i + 1) * 128])
                            for which, dest in ((0, q_sb[i]), (1, k_sb[i])):
                                for qc in range(QC):
                                    ps = mm_ps.tile([128, 512], f32, tag="mm")
                                    for d in range(DT):
                                        mm(ps[:],
                                           wqk_sb[d][:, which * 128:
                                                     (which + 1) * 128],
                                           xt_sb[d][:, qc * 512:(qc + 1) * 512],
                                           start=(d == 0), stop=(d == DT - 1))
                                    nc.vector.tensor_copy(
                                        dest[:, qc * 512:(qc + 1) * 512], ps[:])

                # ---------------- phase 2: attention ----------------
                with ExitStack() as att:
                    p_pool = att.enter_context(tc.tile_pool(name="psb", bufs=3))
                    n_pool = att.enter_context(tc.tile_pool(name="nsb", bufs=4))
                    sc_ps = att.enter_context(
                        tc.tile_pool(name="scps", bufs=1, space="PSUM"))
                    y_ps = att.enter_context(
                        tc.tile_pool(name="yps", bufs=2, space="PSUM"))

                    for i in range(NPAIRS):
                        for qc in range(QC):
                            nkt = 4 * qc + 4   # causal: k-tiles 0 .. 4qc+3
                            yps = [y_ps.tile([65, 512], f32, tag="yt", name=f"yps{i}_{qc}_{h}")
                                   for h in range(2)]
                            for kg in range(nkt // 2):
                                kts = (2 * kg, 2 * kg + 1)
                                sc = sc_ps.tile([128, 2048], f32, tag="sc")
                                pt = p_pool.tile([128, 2048], f32r, tag="p")
                                for sub, kt in enumerate(kts):
                                    j = kt - 4 * qc
                                    lo = max(0, j) * 128
                                    for h in range(2):
                                        base = (2 * sub + h) * 512
                                        mm(sc[:, base + lo:base + 512],
                                           k_sb[i][h * 64:(h + 1) * 64,
                                                   kt * 128:(kt + 1) * 128],
                                           q_sb[i][h * 64:(h + 1) * 64,
                                                   qc * 512 + lo:
                                                   (qc + 1) * 512],
                                           start=True, stop=True)
                                # exp(score / 8)
                                for sub, kt in enumerate(kts):
                                    j = kt - 4 * qc
                                    lo = max(0, j) * 128
                                    src = sc[:].rearrange(
                                        "p (s c) -> p s c", s=4)[
                                        :, 2 * sub:2 * sub + 2, lo:512]
                                    dst = pt[:].rearrange(
                                        "p (s c) -> p s c", s=4)[
                                        :, 2 * sub:2 * sub + 2, lo:512]
                                    nc.scalar.activation(dst, src, AF.Exp,
                                                         scale=0.125)
                                    if j >= 0:   # mask the diagonal band
                                        for h in range(2):
                                            band = pt[
                                                :, (2 * sub + h) * 512 + lo:
                                                (2 * sub + h) * 512 + lo + 128]
                                            nc.vector.tensor_mul(
                                                band, band, tri_sb[:])
                                # AV accumulate (M=65: y dims + sum column)
                                for sub, kt in enumerate(kts):
                                    j = kt - 4 * qc
                                    lo = max(0, j) * 128
                                    for h in range(2):
                                        hl = 2 * i + h
                                        mm(yps[h][:, lo:512],
                                           v_sb[kt][:, hl * 65:hl * 65 + 65],
                                           pt[:, (2 * sub + h) * 512 + lo:
                                              (2 * sub + h) * 512 + 512],
                                           start=(kt == 0),
                                           stop=(kt == nkt - 1))
                            # normalize: y[0:64] * (1 / y[64]); the sum row is
                            # DMA-moved to partition 0 (DVE custom ops only
                            # work at base partition 0), recip'd, broadcast to
                            # 64 partitions by a K=1 matmul, multiplied in, and
                            # the result streamed straight to y_loc in DRAM.
                            for h in range(2):
                                ysc = n_pool.tile([65, 512], f32, tag="ysc")
                                nc.vector.tensor_copy(ysc[:], yps[h][:])
                                srow = n_pool.tile([1, 512], f32, tag="srow")
                                nc.sync.dma_start(srow[:], ysc[64:65, :])
                                rcp = n_pool.tile([1, 512], f32, tag="rcp")
                                nc.vector.reciprocal_approx_fast(
                                    out=rcp[:], in_=srow[:])
                                rb = mm_ps.tile([64, 512], f32, tag="mm")
                                nc.tensor.matmul(
                                    rb[:], ones_sb[0:1, 0:64],
                                    rcp[:], start=True, stop=True)
                                nout = n_pool.tile([64, 512], f32r, tag="nout")
                                nc.vector.tensor_mul(
                                    nout[:], ysc[0:64, :], rb[:])
                                nc.sync.dma_start(
                                    y_loc[(2 * i + h) * 64:
                                          (2 * i + h + 1) * 64,
                                          qc * 512:(qc + 1) * 512], nout[:])

                    # ------------ phase 3: pair AllGather of yT ------------
                    nc.gpsimd.collective_compute(
                        "AllGather", OP.bypass,
                        replica_groups=[[0, 1], [2, 3], [4, 5], [6, 7]],
                        ins=[y_loc[:]], outs=[y_gat[:]])

            # ---------------- phase 4: output projection ----------------
            with ExitStack() as oproj:
                wo_pool = oproj.enter_context(
                    tc.tile_pool(name="wosb", bufs=DT))
                yg_pool = oproj.enter_context(
                    tc.tile_pool(name="ygsb", bufs=DT))
                o_pool = oproj.enter_context(tc.tile_pool(name="osb", bufs=2))

                wo_sb = [wo_pool.tile([128, EHALF], f32r, tag="wo", name=f"wo{d}")
                         for d in range(DT)]
                for d in range(DT):
                    nc.sync.dma_start(wo_sb[d][:],
                                      woT[d * 128:(d + 1) * 128, :])
                yg_sb = [yg_pool.tile([128, S], f32r, tag="yg", name=f"yg{k}")
                         for k in range(DT)]
                for k in range(DT):
                    nc.sync.dma_start(
                        yg_sb[k][:],
                        y_gat[k // 4, (k % 4) * 128:(k % 4 + 1) * 128, :])

                for m in range(EHALF // 128):
                    for tch in range(QC):
                        ps = mm_ps.tile([128, 512], f32, tag="mm")
                        for k in range(DT):
                            mm(ps[:], wo_sb[k][:, m * 128:(m + 1) * 128],
                               yg_sb[k][:, tch * 512:(tch + 1) * 512],
                               start=(k == 0), stop=(k == DT - 1))
                        ob = o_pool.tile([128, 512], f32, tag="o")
                        nc.vector.tensor_copy(ob[:], ps[:])
                        nc.sync.dma_start(
                            outT[m * 128:(m + 1) * 128,
                                 tch * 512:(tch + 1) * 512], ob[:])
    nc.finalize()
    return nc


def _prep_inputs(x, w_in, w_out):
    """Build per-core input maps (host-side sharding)."""
    x = np.ascontiguousarray(x, dtype=np.float32)
    w_in = np.ascontiguousarray(w_in, dtype=np.float32)
    w_out = np.ascontiguousarray(w_out, dtype=np.float32)

    tri = np.triu(np.ones((128, 128), dtype=np.float32))  # 1 where k <= q
    in_maps = []
    for c in range(N_CORES):
        b, g = c // 2, c % 2
        heads = [8 * g + h for h in range(HPC)]
        xTb = np.ascontiguousarray(x[b].T)                       # [D, S]
        # wqkT: cols i*128 -> q rows of heads (8g+2i, 8g+2i+1); then k pairs
        qcols, kcols = [], []
        for i in range(NPAIRS):
            hA, hB = heads[2 * i], heads[2 * i + 1]
            qcols.append(w_in[hA * HD:(hA + 1) * HD, :])
            qcols.append(w_in[hB * HD:(hB + 1) * HD, :])
            kcols.append(w_in[D + hA * HD:D + (hA + 1) * HD, :])
            kcols.append(w_in[D + hB * HD:D + (hB + 1) * HD, :])
        wqkT = np.ascontiguousarray(
            np.concatenate(qcols + kcols, axis=0).T)             # [D, 1024]
        wvT = np.ascontiguousarray(np.concatenate(
            [w_in[2 * D + h * HD:2 * D + (h + 1) * HD, :] for h in heads],
            axis=0).T)                                           # [D, 512]
        woT = np.ascontiguousarray(
            w_out[g * EHALF:(g + 1) * EHALF, :].T)               # [D, 512]
        in_maps.append({
            "xT": xTb, "wqkT": wqkT, "wvT": wvT, "woT": woT, "tri": tri,
        })
    return in_maps


def kernel(x, w_in, w_out):
    global _PROG
    from concourse.bass_utils import run_bass_kernel_spmd

    if _PROG is None:
        _PROG = _build_program()
    in_maps = _prep_inputs(x, w_in, w_out)
    res = run_bass_kernel_spmd(_PROG, in_maps, list(range(N_CORES)))

    out = np.empty((B, S, D), dtype=np.float32)
    for c in range(N_CORES):
        b, g = c // 2, c % 2
        out[b, :, g * EHALF:(g + 1) * EHALF] = res.results[c]["outT"].T
    return out


# revision 15
# speedup vs baseline: 1.7006x; 1.7006x over previous
"""Causal self-attention on 8 Trainium2 NeuronCores (Bass/Tile).

Problem: x[4, 2048, 1024], w_in[3072, 1024], w_out[1024, 1024], 16 heads.
    qkv = x @ w_in.T ; per-(b,h) causal softmax attention ; out = y @ w_out.T

Sharding (SPMD — one program, per-core input data):
    core c  ->  batch b = c // 2, head-group g = c % 2 (heads 8g .. 8g+7).
    Each core projects q/k/v for its 8 heads of its batch and runs causal
    attention for them.  The pair (2b, 2b+1) AllGathers the two head-group
    halves of yT (chunked per head-pair so it overlaps attention), then each
    core computes the output projection for half of the output features
    (core even: e_out 0..511, odd: 512..1023) over all 2048 tokens of its
    batch.  The host concatenates.

Everything on-chip is kept feature-major ("T" = contraction dim on SBUF
partitions) so no on-device transposes are needed:
    xT [D, S] (host-transposed), qT/kT per head-pair [128, S],
    scoresT [k, q], yT [e, t], outT [e_out, t] (host-transposed back).
Head-pair q/k projections are interleaved with that pair's attention so the
PE stays dense while the ACT engine works through the exps.  Softmax
denominators come from a ones-column appended to V (the AV matmul has
M = 65); normalization is a fast DVE reciprocal on the sum row (moved to
partition 0 by a small DMA — custom DVE ops only work at base partition 0)
+ a K=1 matmul broadcast + one multiply, streamed straight to DRAM.
Matmuls run as float32r (reduced-precision fp32, full PE rate at N >= 256).
"""

import sys

for _p in ("/opt/trn_rl_repo",):
    if _p not in sys.path:
        sys.path.insert(0, _p)

import numpy as np

B, S, D = 4, 2048, 1024
H, HD = 16, 64
N_CORES = 8
HPC = 8            # heads per core
NPAIRS = HPC // 2  # head pairs per core
QC = S // 512      # q-chunks per head
TT = S // 128      # token tiles
DT = D // 128      # feature (d) tiles
EHALF = D // 2     # output features per core

_PROG = None       # cached compiled program


def _build_program():
    import concourse.bass as bass
    from concourse import bacc
    import concourse.tile as tile
    import concourse.mybir as mybir
    from contextlib import ExitStack

    f32 = mybir.dt.float32
    f32r = mybir.dt.float32r
    AF = mybir.ActivationFunctionType
    OP = mybir.AluOpType

    nc = bacc.Bacc("TRN2", target_bir_lowering=False, debug=False,
                   num_devices=N_CORES)

    xT = nc.dram_tensor("xT", [D, S], f32r, kind="ExternalInput").ap()
    wqkT = nc.dram_tensor("wqkT", [D, 2 * HPC * HD], f32r,
                          kind="ExternalInput").ap()
    wvT = nc.dram_tensor("wvT", [D, HPC * HD], f32r, kind="ExternalInput").ap()
    woT = nc.dram_tensor("woT", [D, EHALF], f32r, kind="ExternalInput").ap()
    tri = nc.dram_tensor("tri", [128, 128], f32, kind="ExternalInput").ap()
    outT = nc.dram_tensor("outT", [EHALF, S], f32, kind="ExternalOutput").ap()

    y_loc = nc.dram_tensor("y_loc", [HPC * HD, S], f32r)
    y_gat = [nc.dram_tensor(f"y_gat{i}", [2, 128, S], f32r)
             for i in range(NPAIRS)]

    with tile.TileContext(nc) as tc:
        def mm(out, lhsT, rhs, start, stop):
            nc.tensor.matmul(out, lhsT, rhs, start=start, stop=stop)

        with ExitStack() as perm:
            const_pool = perm.enter_context(tc.tile_pool(name="const", bufs=1))
            v_pool = perm.enter_context(tc.tile_pool(name="vsb", bufs=TT))
            mm_ps = perm.enter_context(
                tc.tile_pool(name="mmps", bufs=2, space="PSUM"))

            tri_sb = const_pool.tile([128, 128], f32, tag="tri")
            nc.sync.dma_start(tri_sb[:], tri[:])
            ones_sb = const_pool.tile([128, 64], f32, tag="ones")
            nc.gpsimd.memset(ones_sb[:], 1.0)
            onesr_sb = const_pool.tile([1, 64], f32r, tag="onesr")
            nc.vector.tensor_copy(onesr_sb[:], ones_sb[0:1, :])

            # v_sb[t]: [128, 8*65] — per head 64 v-columns + a ones column
            v_sb = [v_pool.tile([128, HPC * (HD + 1)], f32r, tag="v",
                                name=f"v{t}") for t in range(TT)]

            with ExitStack() as att_scope:
                qk_pool = att_scope.enter_context(
                    tc.tile_pool(name="qksb", bufs=4))
                xt_pool = att_scope.enter_context(
                    tc.tile_pool(name="xtsb", bufs=DT))
                wqk_pool = att_scope.enter_context(
                    tc.tile_pool(name="wqksb", bufs=2 * DT))
                p_pool = att_scope.enter_context(
                    tc.tile_pool(name="psb", bufs=3))
                n_pool = att_scope.enter_context(
                    tc.tile_pool(name="nsb", bufs=2))
                sc_ps = att_scope.enter_context(
                    tc.tile_pool(name="scps", bufs=2, space="PSUM"))
                y_ps = att_scope.enter_context(
                    tc.tile_pool(name="yps", bufs=2, space="PSUM"))

                xt_sb = [xt_pool.tile([128, S], f32r, tag="xt", name=f"xt{d}")
                         for d in range(DT)]
                for d in range(DT):
                    nc.sync.dma_start(xt_sb[d][:], xT[d * 128:(d + 1) * 128, :])

                # ---- v projection: v[t, e] accumulated over d ----
                with tc.tile_pool(name="wvsb", bufs=DT) as wv_pool:
                    wv_sb = [wv_pool.tile([128, HPC * HD], f32r, tag="wv",
                                          name=f"wv{d}") for d in range(DT)]
                    for d in range(DT):
                        nc.sync.dma_start(wv_sb[d][:],
                                          wvT[d * 128:(d + 1) * 128, :])
                    for t in range(TT):
                        ps = mm_ps.tile([128, 512], f32, tag="mm")
                        for d in range(DT):
                            mm(ps[:], xt_sb[d][:, t * 128:(t + 1) * 128],
                               wv_sb[d][:], start=(d == 0), stop=(d == DT - 1))
                        vdst = v_sb[t][:].rearrange(
                            "p (h e) -> p h e", h=HPC)[:, :, 0:HD]
                        vsrc = ps[:].rearrange("p (h e) -> p h e", h=HPC)
                        nc.vector.tensor_copy(vdst, vsrc)
                        nc.vector.tensor_copy(
                            v_sb[t][:].rearrange(
                                "p (h e) -> p h e", h=HPC)[:, :, HD:HD + 1],
                            ones_sb[:, 0:HPC].unsqueeze(-1))

                # ---- per pair: q/k projection then attention ----
                for i in range(NPAIRS):
                    wqk_sb = [wqk_pool.tile([128, 256], f32r, tag="wqk",
                                            name=f"wqk{i}_{d}")
                              for d in range(DT)]
                    for d in range(DT):
                        nc.sync.dma_start(
                            wqk_sb[d][:, 0:128],
                            wqkT[d * 128:(d + 1) * 128, i * 128:(i + 1) * 128])
                        nc.sync.dma_start(
                            wqk_sb[d][:, 128:256],
                            wqkT[d * 128:(d + 1) * 128,
                                 (NPAIRS + i) * 128:(NPAIRS + i + 1) * 128])
                    q_sb = qk_pool.tile([128, S], f32r, tag="qk", name=f"q{i}")
                    k_sb = qk_pool.tile([128, S], f32r, tag="qk", name=f"k{i}")
                    for which, dest in ((0, q_sb), (1, k_sb)):
                        for qc in range(QC):
                            ps = mm_ps.tile([128, 512], f32, tag="mm")
                            for d in range(DT):
                                mm(ps[:],
                                   wqk_sb[d][:, which * 128:(which + 1) * 128],
                                   xt_sb[d][:, qc * 512:(qc + 1) * 512],
                                   start=(d == 0), stop=(d == DT - 1))
                            nc.vector.tensor_copy(
                                dest[:, qc * 512:(qc + 1) * 512], ps[:])

                    # ---- attention for this pair ----
                    for qc in range(QC):
                        nkt = 4 * qc + 4   # causal: k-tiles 0 .. 4qc+3
                        yps = [y_ps.tile([65, 512], f32, tag="yt",
                                         name=f"yps{i}_{qc}_{h}")
                               for h in range(2)]
                        for kt in range(nkt):
                            j = kt - 4 * qc
                            lo = max(0, j) * 128
                            sc = sc_ps.tile([128, 1024], f32, tag="sc")
                            pt = p_pool.tile([128, 1024], f32r, tag="p")
                            for h in range(2):
                                mm(sc[:, h * 512 + lo:(h + 1) * 512],
                                   k_sb[h * 64:(h + 1) * 64,
                                        kt * 128:(kt + 1) * 128],
                                   q_sb[h * 64:(h + 1) * 64,
                                        qc * 512 + lo:(qc + 1) * 512],
                                   start=True, stop=True)
                            # exp(score / 8) for both heads in one ACT call
                            src = sc[:].rearrange("p (s c) -> p s c", s=2)[
                                :, :, lo:512]
                            dst = pt[:].rearrange("p (s c) -> p s c", s=2)[
                                :, :, lo:512]
                            nc.scalar.activation(dst, src, AF.Exp, scale=0.125)
                            if j >= 0:   # mask the diagonal band
                                for h in range(2):
                                    band = pt[:, h * 512 + lo:
                                              h * 512 + lo + 128]
                                    nc.vector.tensor_mul(band, band, tri_sb[:])
                            for h in range(2):
                                hl = 2 * i + h
                                mm(yps[h][:, lo:512],
                                   v_sb[kt][:, hl * 65:hl * 65 + 65],
                                   pt[:, h * 512 + lo:(h + 1) * 512],
                                   start=(kt == 0), stop=(kt == nkt - 1))
                        # normalize: y[0:64] * (1 / y[64]) and stream to DRAM
                        for h in range(2):
                            ysc = n_pool.tile([65, 512], f32, tag="ysc")
                            nc.vector.tensor_copy(ysc[:], yps[h][:])
                            srow = n_pool.tile([1, 512], f32, tag="srow")
                            nc.sync.dma_start(srow[:], ysc[64:65, :])
                            rcp = n_pool.tile([1, 512], f32, tag="rcp")
                            nc.vector.reciprocal_approx_fast(
                                out=rcp[:], in_=srow[:])
                            rcpr = n_pool.tile([1, 512], f32r, tag="rcpr")
                            nc.vector.tensor_copy(rcpr[:], rcp[:])
                            rb = mm_ps.tile([64, 512], f32, tag="mm")
                            mm(rb[:], onesr_sb[:], rcpr[:],
                               start=True, stop=True)
                            nout = n_pool.tile([64, 512], f32r, tag="nout")
                            nc.vector.tensor_mul(nout[:], ysc[0:64, :], rb[:])
                            nc.sync.dma_start(
                                y_loc[(2 * i + h) * 64:(2 * i + h + 1) * 64,
                                      qc * 512:(qc + 1) * 512], nout[:])

                    # ---- chunked pair AllGather for this head-pair ----
                    nc.gpsimd.collective_compute(
                        "AllGather", OP.bypass,
                        replica_groups=[[0, 1], [2, 3], [4, 5], [6, 7]],
                        ins=[y_loc[i * 128:(i + 1) * 128, :]],
                        outs=[y_gat[i][:]])

            # ---------------- output projection ----------------
            with ExitStack() as oproj:
                wo_pool = oproj.enter_context(
                    tc.tile_pool(name="wosb", bufs=DT))
                yg_pool = oproj.enter_context(
                    tc.tile_pool(name="ygsb", bufs=DT))
                o_pool = oproj.enter_context(tc.tile_pool(name="osb", bufs=2))

                wo_sb = [wo_pool.tile([128, EHALF], f32r, tag="wo",
                                      name=f"wo{d}") for d in range(DT)]
                for d in range(DT):
                    nc.sync.dma_start(wo_sb[d][:],
                                      woT[d * 128:(d + 1) * 128, :])
                yg_sb = [yg_pool.tile([128, S], f32r, tag="yg", name=f"yg{k}")
                         for k in range(DT)]
                for k in range(DT):
                    nc.sync.dma_start(yg_sb[k][:], y_gat[k % NPAIRS][k // NPAIRS])

                for m in range(EHALF // 128):
                    for tch in range(QC):
                        ps = mm_ps.tile([128, 512], f32, tag="mm")
                        for k in range(DT):
                            mm(ps[:], wo_sb[k][:, m * 128:(m + 1) * 128],
                               yg_sb[k][:, tch * 512:(tch + 1) * 512],
                               start=(k == 0), stop=(k == DT - 1))
                        ob = o_pool.tile([128, 512], f32, tag="o")
                        nc.vector.tensor_copy(ob[:], ps[:])
                        nc.sync.dma_start(
                            outT[m * 128:(m + 1) * 128,
                                 tch * 512:(tch + 1) * 512], ob[:])
    nc.finalize()
    return nc


def _prep_inputs(x, w_in, w_out):
    """Build per-core input maps (host-side sharding)."""
    x = np.ascontiguousarray(x, dtype=np.float32)
    w_in = np.ascontiguousarray(w_in, dtype=np.float32)
    w_out = np.ascontiguousarray(w_out, dtype=np.float32)

    tri = np.triu(np.ones((128, 128), dtype=np.float32))  # 1 where k <= q
    in_maps = []
    for c in range(N_CORES):
        b, g = c // 2, c % 2
        heads = [8 * g + h for h in range(HPC)]
        xTb = np.ascontiguousarray(x[b].T)                       # [D, S]
        # wqkT: cols i*128 -> q rows of heads (8g+2i, 8g+2i+1); then k pairs
        qcols, kcols = [], []
        for i in range(NPAIRS):
            hA, hB = heads[2 * i], heads[2 * i + 1]
            qcols.append(w_in[hA * HD:(hA + 1) * HD, :])
            qcols.append(w_in[hB * HD:(hB + 1) * HD, :])
            kcols.append(w_in[D + hA * HD:D + (hA + 1) * HD, :])
            kcols.append(w_in[D + hB * HD:D + (hB + 1) * HD, :])
        wqkT = np.ascontiguousarray(
            np.concatenate(qcols + kcols, axis=0).T)             # [D, 1024]
        wvT = np.ascontiguousarray(np.concatenate(
            [w_in[2 * D + h * HD:2 * D + (h + 1) * HD, :] for h in heads],
            axis=0).T)                                           # [D, 512]
        woT = np.ascontiguousarray(
            w_out[g * EHALF:(g + 1) * EHALF, :].T)               # [D, 512]
        in_maps.append({
            "xT": xTb, "wqkT": wqkT, "wvT": wvT, "woT": woT, "tri": tri,
        })
    return in_maps


def kernel(x, w_in, w_out):
    global _PROG
    from concourse.bass_utils import run_bass_kernel_spmd

    if _PROG is None:
        _PROG = _build_program()
    in_maps = _prep_inputs(x, w_in, w_out)
    res = run_bass_kernel_spmd(_PROG, in_maps, list(range(N_CORES)))

    out = np.empty((B, S, D), dtype=np.float32)
    for c in range(N_CORES):
        b, g = c // 2, c % 2
        out[b, :, g * EHALF:(g + 1) * EHALF] = res.results[c]["outT"].T
    return out
